# revision 9
# baseline (speedup 1.0000x reference)
"""Causal self-attention (B=4, T=2048, C=2048, H=16, RoPE) on 8 trn2 NeuronCores.

Sharding: data-parallel over B (4) x tensor-parallel over heads (2 groups of 8).
Core c handles batch b = c // 2, heads [8*(c%2), 8*(c%2)+8). Each core computes
its partial c_proj output; the host sums the two partials per batch element.

v4 (scheduling rework of v3; same bf16 numerics):
  - Softmax denominators accumulate into ONE (8,512) PSUM tile per chunk
    (each head's ones-matmuls write partition row h), so a single batched
    reciprocal per chunk replaces 8 serial 3.3us DVE reciprocals, and no
    per-head PE<->DVE round trip gates the psl pool rotation.
  - psy is copied (unnormalized, on DVE, one head delayed to not block mask
    muls) into the ytc tile; normalization (recip+broadcast+mul) for chunk
    N runs at the top of chunk N+1, and c_proj for chunk N runs as one
    contiguous 8-chain block at chunk N+1's head-2 slot -- PE never waits
    on the normalization chain.
  - DMA queues: x^T chunk halves + masks + wp on Sync; all weight loads +
    trig on GPSIMD (interleaved so wk/trig land just in time); RoPE
    half-swaps on the Vector queue (self-ordered with the rope muls);
    output writes on GPSIMD. Pass-1 wq prefetches into a 4th weight buffer
    during pass 0.
"""

import sys

if "/opt/trn_rl_repo" not in sys.path:
    sys.path.insert(0, "/opt/trn_rl_repo")

import numpy as np

B, T, C = 4, 2048, 2048
H, NH = 16, 8  # total heads, heads per core
D = C // H  # 128
N_CORES = 8
ROPE_THETA = 10000.0
NCT = C // 128  # 16 contraction tiles
NTC = T // 512  # 4 t-chunks
NTB = T // 128  # 16 t/s blocks
SCALE = float(D) ** -0.5

_CACHE = {}


def _build_module():
    import concourse.bacc as bacc
    import concourse.tile as tile
    from concourse import mybir

    f32 = mybir.dt.float32
    bf16 = mybir.dt.bfloat16

    nc = bacc.Bacc("TRN2", target_bir_lowering=False, debug=False,
                   num_devices=N_CORES)

    xt_h = nc.dram_tensor("xt_h", [NTC, 128, NCT, 512], bf16,
                          kind="ExternalInput")
    wq_h = nc.dram_tensor("wq_h", [2, 4, 128, NCT, D], bf16,
                          kind="ExternalInput")
    wk_h = nc.dram_tensor("wk_h", [2, 4, 128, NCT, D], bf16,
                          kind="ExternalInput")
    wv_h = nc.dram_tensor("wv_h", [2, 128, NCT, 512], bf16,
                          kind="ExternalInput")
    wp_h = nc.dram_tensor("wp_h", [2, 128, 4, C], bf16, kind="ExternalInput")
    trig_c = nc.dram_tensor("trig_c", [128, T], bf16, kind="ExternalInput")
    trig_s = nc.dram_tensor("trig_s", [128, T], bf16, kind="ExternalInput")
    masks = nc.dram_tensor("masks", [128, 4, 512], bf16, kind="ExternalInput")
    out = nc.dram_tensor("out", [T, C], f32, kind="ExternalOutput")

    with tile.TileContext(nc) as tc:
        with tc.tile_pool(name="singles", bufs=1) as singles:
            q_all = singles.tile([128, NH, T], bf16)   # (d, h, t)
            k_all = singles.tile([128, NH, T], bf16)   # (d, h, t)
            v_all = singles.tile([128, NTB, NH * D], bf16)  # (t%128, sb, d)
            ones_f = singles.tile([128, 1], f32)
            ones_t = singles.tile([128, 1], bf16)

            # ---------------- Phase 1: fused QKV projections + RoPE --------
            with tc.tile_pool(name="wpool", bufs=4) as wpool, \
                 tc.tile_pool(name="trigp", bufs=1) as trigp, \
                 tc.tile_pool(name="xtp", bufs=2) as xtp, \
                 tc.tile_pool(name="ropea", bufs=2) as ropea, \
                 tc.tile_pool(name="ropeb", bufs=2) as ropeb, \
                 tc.tile_pool(name="ropec", bufs=2) as ropec, \
                 tc.tile_pool(name="psqk", bufs=4, space="PSUM") as psqkp, \
                 tc.tile_pool(name="psv", bufs=3, space="PSUM") as psvp:
                trig_c_t = trigp.tile([128, T], bf16)
                trig_s_t = trigp.tile([128, T], bf16)
                nc.vector.memset(ones_f[:], 1.0)
                nc.vector.tensor_copy(ones_t[:], ones_f[:])

                def load_w(tile_, src):
                    for hl in range(4):
                        nc.gpsimd.dma_start(out=tile_[:, hl], in_=src[hl])

                wq_next = wpool.tile([128, 4, NCT, D], bf16, tag="w")
                # interleaved first-pass loads: wq[h0], wk[h0], trig, rest
                nc.gpsimd.dma_start(out=wq_next[:, 0], in_=wq_h[0, 0])
                for half in range(2):
                    wq_t = wq_next
                    wk_t = wpool.tile([128, 4, NCT, D], bf16, tag="w")
                    if half == 0:
                        nc.gpsimd.dma_start(out=wk_t[:, 0], in_=wk_h[0, 0])
                        nc.gpsimd.dma_start(out=trig_c_t[:], in_=trig_c[:])
                        nc.gpsimd.dma_start(out=trig_s_t[:], in_=trig_s[:])
                        for hl in range(1, 4):
                            nc.gpsimd.dma_start(out=wq_t[:, hl],
                                                in_=wq_h[0, hl])
                            nc.gpsimd.dma_start(out=wk_t[:, hl],
                                                in_=wk_h[0, hl])
                    else:
                        load_w(wk_t, wk_h[1])
                    wv_t = wpool.tile([128, NCT, 512], bf16, tag="w")
                    nc.gpsimd.dma_start(out=wv_t[:], in_=wv_h[half])
                    for tci in range(NTC):
                        ts_ = slice(tci * 512, (tci + 1) * 512)
                        xt_t = xtp.tile([128, NCT, 512], bf16, tag="xt")
                        nsp = 4 if (half == 0 and tci == 0) else 2
                        for sp in range(nsp):
                            cs = slice(sp * NCT // nsp, (sp + 1) * NCT // nsp)
                            nc.sync.dma_start(out=xt_t[:, cs, :],
                                              in_=xt_h[tci, :, cs, :])
                        if half == 0 and tci == 2:
                            # prefetch pass-1 wq into the free 4th buffer
                            wq_next = wpool.tile([128, 4, NCT, D], bf16,
                                                 tag="w")
                            load_w(wq_next, wq_h[1])
                        for hl in range(4):
                            h = half * 4 + hl
                            lsl = slice(hl * D, (hl + 1) * D)
                            for qk in range(2):
                                w_t = wq_t if qk == 0 else wk_t
                                dst = q_all if qk == 0 else k_all
                                ps = psqkp.tile([128, 512], f32, tag="psqk")
                                for ct in range(NCT):
                                    nc.tensor.matmul(
                                        ps[:], w_t[:, hl, ct, :],
                                        xt_t[:, ct, :],
                                        start=(ct == 0), stop=(ct == NCT - 1))
                                qsb = ropea.tile([128, 512], bf16, tag="qsb")
                                nc.scalar.copy(qsb[:], ps[:])
                                qsw = ropeb.tile([128, 512], bf16, tag="qsw")
                                nc.scalar.dma_start(out=qsw[0:64, :],
                                                    in_=qsb[64:128, :])
                                nc.scalar.dma_start(out=qsw[64:128, :],
                                                    in_=qsb[0:64, :])
                                rot = ropec.tile([128, 512], bf16, tag="rot")
                                nc.vector.tensor_mul(rot[:], qsw[:],
                                                     trig_s_t[:, ts_])
                                nc.vector.tensor_mul(qsb[:], qsb[:],
                                                     trig_c_t[:, ts_])
                                nc.vector.tensor_add(dst[:, h, ts_],
                                                     qsb[:], rot[:])
                        for sb in range(4):
                            ssl = slice(sb * 128, (sb + 1) * 128)
                            psv = psvp.tile([128, 512], f32, tag="psv")
                            for ct in range(NCT):
                                nc.tensor.matmul(
                                    psv[:], xt_t[:, ct, ssl], wv_t[:, ct, :],
                                    start=(ct == 0), stop=(ct == NCT - 1))
                            nc.scalar.copy(
                                v_all[:, tci * 4 + sb,
                                      half * 512:(half + 1) * 512],
                                psv[:])

            # ---------------- Phase 2: attention + deferred fused c_proj ---
            with tc.tile_pool(name="wppool", bufs=2) as wppool, \
                 tc.tile_pool(name="maskp", bufs=1) as maskp, \
                 tc.tile_pool(name="ptp", bufs=8) as ptp, \
                 tc.tile_pool(name="ytcp", bufs=2) as ytcp, \
                 tc.tile_pool(name="recp", bufs=2) as recp, \
                 tc.tile_pool(name="rbp", bufs=2) as rbp, \
                 tc.tile_pool(name="osbp", bufs=2) as osbp, \
                 tc.tile_pool(name="pss", bufs=2, space="PSUM") as pssp, \
                 tc.tile_pool(name="psy", bufs=2, space="PSUM") as psyp, \
                 tc.tile_pool(name="psl", bufs=2, space="PSUM") as pslp:
                masks_t = maskp.tile([128, 4, 512], bf16)
                nc.sync.dma_start(out=masks_t[:], in_=masks[:])
                wp_ts = []
                for half in range(2):
                    wp_t = wppool.tile([128, 4, C], bf16, tag="wp")
                    nc.sync.dma_start(out=wp_t[:], in_=wp_h[half])
                    wp_ts.append(wp_t)

                def emit_s_exp(tci, h):
                    ts_ = slice(tci * 512, (tci + 1) * 512)
                    jmax = 4 * tci + 3
                    pts = []
                    for jp in range((jmax + 1) // 2):
                        pss = pssp.tile([128, 2, 512], f32, tag="pss")
                        for i in range(2):
                            j = 2 * jp + i
                            nc.tensor.matmul(
                                pss[:, i, :],
                                k_all[:, h, j * 128:(j + 1) * 128],
                                q_all[:, h, ts_],
                                start=True, stop=True)
                        pt = ptp.tile([128, 2, 512], bf16, tag="pt")
                        nc.scalar.activation(
                            pt[:], pss[:],
                            mybir.ActivationFunctionType.Exp, scale=SCALE)
                        for i in range(2):
                            j = 2 * jp + i
                            if j >= 4 * tci:
                                nc.vector.tensor_mul(
                                    pt[:, i, :], pt[:, i, :],
                                    masks_t[:, j - 4 * tci, :])
                        pts.append(pt)
                    return pts

                def emit_pv_norm(tci, h, pts, ytc):
                    jmax = 4 * tci + 3
                    psy = psyp.tile([128, 512], f32, tag="psy")
                    psl = pslp.tile([1, 512], f32, tag="psl")
                    for jp in range((jmax + 1) // 2):
                        pt = pts[jp]
                        for i in range(2):
                            j = 2 * jp + i
                            nc.tensor.matmul(
                                psy[:], v_all[:, j, h * D:(h + 1) * D],
                                pt[:, i, :],
                                start=(j == 0), stop=(j == jmax))
                            nc.tensor.matmul(
                                psl[:], ones_t[:], pt[:, i, :],
                                start=(j == 0), stop=(j == jmax))
                    rec = recp.tile([1, 512], f32, tag="rec")
                    nc.vector.reciprocal(rec[:], psl[:])
                    rb = rbp.tile([128, 512], f32, tag="rb")
                    nc.gpsimd.partition_broadcast(rb[:], rec[:])
                    nc.vector.tensor_mul(ytc[:, h, :], psy[:], rb[:])

                def emit_cproj_block(ytc_src, tci_src):
                    for tb in range(4):
                        osb = osbp.tile([128, C], f32, tag="osb")
                        tls = slice(tb * 128, (tb + 1) * 128)
                        for ecp in range(2):
                            pso = pssp.tile([128, 2, 512], f32, tag="pss")
                            for i in range(2):
                                ec = 2 * ecp + i
                                es = slice(ec * 512, (ec + 1) * 512)
                                for h in range(NH):
                                    nc.tensor.matmul(
                                        pso[:, i, :], ytc_src[:, h, tls],
                                        wp_ts[h // 4][:, h % 4, es],
                                        start=(h == 0), stop=(h == NH - 1))
                            nc.vector.tensor_copy(
                                osb[:, ecp * 1024:(ecp + 1) * 1024], pso[:])
                        tb_glob = tci_src * 4 + tb
                        tbs = slice(tb_glob * 128, (tb_glob + 1) * 128)
                        nc.gpsimd.dma_start(out=out[tbs, :], in_=osb[:])

                seq = [1, 0, 2, 3]
                ytc_prev = None
                tci_prev = None
                for si, tci in enumerate(seq):
                    ytc = ytcp.tile([128, NH, 512], bf16, tag="ytc")
                    pipelined = si <= 1
                    pend = None  # (h, pts) awaiting PV in pipelined mode
                    for h in range(NH):
                        pts = emit_s_exp(tci, h)
                        if pipelined:
                            if pend is not None:
                                emit_pv_norm(tci, pend[0], pend[1], ytc)
                            pend = (h, pts)
                        else:
                            emit_pv_norm(tci, h, pts, ytc)
                        if h == 2 and si > 0:
                            emit_cproj_block(ytc_prev, tci_prev)
                    if pipelined and pend is not None:
                        emit_pv_norm(tci, pend[0], pend[1], ytc)
                    ytc_prev = ytc
                    tci_prev = tci
                # tail: c_proj for the last chunk in sequence
                emit_cproj_block(ytc_prev, tci_prev)

    nc.compile()
    return nc


def _prep_inputs(x, w_attn, w_proj):
    """Build the 8 per-core input maps (host-side shard + bf16 + relayout)."""
    import ml_dtypes
    bf16 = ml_dtypes.bfloat16

    perm = np.concatenate([np.arange(0, D, 2), np.arange(1, D, 2)])

    inv = 1.0 / np.power(
        np.float32(ROPE_THETA),
        np.arange(0, D, 2, dtype=np.float32) / np.float32(D))
    pos = np.arange(T, dtype=np.float32)
    freqs = pos[:, None] * inv[None, :]  # (T, 64)
    cos_t = np.cos(freqs).T.astype(np.float32)  # (64, T)
    sin_t = np.sin(freqs).T.astype(np.float32)
    trig_c = np.concatenate([cos_t, cos_t], axis=0).astype(bf16)  # (128, T)
    trig_s = np.concatenate([-sin_t, sin_t], axis=0).astype(bf16)

    masks = np.zeros((128, 4, 512), dtype=np.float32)
    tri = (np.arange(128)[None, :] >= np.arange(128)[:, None]).astype(np.float32)
    for r in range(4):
        masks[:, r, r * 128:(r + 1) * 128] = tri
        masks[:, r, (r + 1) * 128:] = 1.0
    masks = masks.astype(bf16)

    wq_full = w_attn[:, 0:C].reshape(C, H, D)
    wk_full = w_attn[:, C:2 * C].reshape(C, H, D)

    in_maps = []
    for core in range(N_CORES):
        b, g = core // 2, core % 2
        hsel = slice(g * NH, (g + 1) * NH)
        xt_t = np.ascontiguousarray(
            x[b].reshape(NTC, 512, NCT, 128).transpose(0, 3, 2, 1)
        ).astype(bf16)
        wqc = wq_full[:, hsel, :][:, :, perm].reshape(C, NH * D)
        wkc = wk_full[:, hsel, :][:, :, perm].reshape(C, NH * D)
        wq_t = np.ascontiguousarray(
            wqc.reshape(NCT, 128, 2, 4, D).transpose(2, 3, 1, 0, 4)
        ).astype(bf16)
        wk_t = np.ascontiguousarray(
            wkc.reshape(NCT, 128, 2, 4, D).transpose(2, 3, 1, 0, 4)
        ).astype(bf16)
        wvc = w_attn[:, 2 * C + g * NH * D: 2 * C + (g + 1) * NH * D]
        wv_t = np.ascontiguousarray(
            wvc.reshape(NCT, 128, 2, 512).transpose(2, 1, 0, 3)).astype(bf16)
        wpc = w_proj[g * NH * D:(g + 1) * NH * D, :]
        wp_t = np.ascontiguousarray(
            wpc.reshape(2, 4, 128, C).transpose(0, 2, 1, 3)).astype(bf16)
        in_maps.append({
            "xt_h": xt_t,
            "wq_h": wq_t,
            "wk_h": wk_t,
            "wv_h": wv_t,
            "wp_h": wp_t,
            "trig_c": trig_c,
            "trig_s": trig_s,
            "masks": masks,
        })
    return in_maps


def _get_module():
    if "nc" not in _CACHE:
        _CACHE["nc"] = _build_module()
    return _CACHE["nc"]


def run_sharded(x, w_attn, w_proj, trace=False):
    """Run on 8 cores; returns BassKernelResults."""
    from concourse.bass_utils import run_bass_kernel_spmd
    nc = _get_module()
    in_maps = _prep_inputs(np.asarray(x), np.asarray(w_attn), np.asarray(w_proj))
    res = run_bass_kernel_spmd(nc, in_maps, core_ids=list(range(N_CORES)),
                               trace=trace)
    return res


def kernel(x, w_attn, w_proj):
    x = np.asarray(x, dtype=np.float32)
    res = run_sharded(x, w_attn, w_proj, trace=False)
    outs = [r["out"] for r in res.results]
    full = np.empty((B, T, C), dtype=np.float32)
    for b in range(B):
        full[b] = outs[2 * b].astype(np.float32) + outs[2 * b + 1].astype(np.float32)
    return full


# revision 10
# speedup vs baseline: 1.0119x; 1.0119x over previous
"""Causal self-attention (B=4, T=2048, C=2048, H=16, RoPE) on 8 trn2 NeuronCores.

Sharding: data-parallel over B (4) x tensor-parallel over heads (2 groups of 8).
Core c handles batch b = c // 2, heads [8*(c%2), 8*(c%2)+8). Each core computes
its partial c_proj output; the host sums the two partials per batch element.

v4 (scheduling rework of v3; same bf16 numerics):
  - Softmax denominators accumulate into ONE (8,512) PSUM tile per chunk
    (each head's ones-matmuls write partition row h), so a single batched
    reciprocal per chunk replaces 8 serial 3.3us DVE reciprocals, and no
    per-head PE<->DVE round trip gates the psl pool rotation.
  - psy is copied (unnormalized, on DVE, one head delayed to not block mask
    muls) into the ytc tile; normalization (recip+broadcast+mul) for chunk
    N runs at the top of chunk N+1, and c_proj for chunk N runs as one
    contiguous 8-chain block at chunk N+1's head-2 slot -- PE never waits
    on the normalization chain.
  - DMA queues: x^T chunk halves + masks + wp on Sync; all weight loads +
    trig on GPSIMD (interleaved so wk/trig land just in time); RoPE
    half-swaps on the Vector queue (self-ordered with the rope muls);
    output writes on GPSIMD. Pass-1 wq prefetches into a 4th weight buffer
    during pass 0.
"""

import sys

if "/opt/trn_rl_repo" not in sys.path:
    sys.path.insert(0, "/opt/trn_rl_repo")

import numpy as np

B, T, C = 4, 2048, 2048
H, NH = 16, 8  # total heads, heads per core
D = C // H  # 128
N_CORES = 8
ROPE_THETA = 10000.0
NCT = C // 128  # 16 contraction tiles
NTC = T // 512  # 4 t-chunks
NTB = T // 128  # 16 t/s blocks
SCALE = float(D) ** -0.5

_CACHE = {}


def _build_module():
    import concourse.bacc as bacc
    import concourse.tile as tile
    from concourse import mybir

    f32 = mybir.dt.float32
    bf16 = mybir.dt.bfloat16

    nc = bacc.Bacc("TRN2", target_bir_lowering=False, debug=False,
                   num_devices=N_CORES)

    xt_h = nc.dram_tensor("xt_h", [NTC, 128, NCT, 512], bf16,
                          kind="ExternalInput")
    wq_h = nc.dram_tensor("wq_h", [2, 4, 128, NCT, D], bf16,
                          kind="ExternalInput")
    wk_h = nc.dram_tensor("wk_h", [2, 4, 128, NCT, D], bf16,
                          kind="ExternalInput")
    wv_h = nc.dram_tensor("wv_h", [2, 128, NCT, 512], bf16,
                          kind="ExternalInput")
    wp_h = nc.dram_tensor("wp_h", [2, 128, 4, C], bf16, kind="ExternalInput")
    trig_c = nc.dram_tensor("trig_c", [128, T], bf16, kind="ExternalInput")
    trig_s = nc.dram_tensor("trig_s", [128, T], bf16, kind="ExternalInput")
    masks = nc.dram_tensor("masks", [128, 4, 512], bf16, kind="ExternalInput")
    out = nc.dram_tensor("out", [T, C], f32, kind="ExternalOutput")

    with tile.TileContext(nc) as tc:
        with tc.tile_pool(name="singles", bufs=1) as singles:
            q_all = singles.tile([128, NH, T], bf16)   # (d, h, t)
            k_all = singles.tile([128, NH, T], bf16)   # (d, h, t)
            v_all = singles.tile([128, NTB, NH * D], bf16)  # (t%128, sb, d)
            ones_f = singles.tile([128, 1], f32)
            ones_t = singles.tile([128, 1], bf16)

            # ---------------- Phase 1: fused QKV projections + RoPE --------
            with tc.tile_pool(name="wpool", bufs=4) as wpool, \
                 tc.tile_pool(name="trigp", bufs=1) as trigp, \
                 tc.tile_pool(name="xtp", bufs=2) as xtp, \
                 tc.tile_pool(name="ropea", bufs=2) as ropea, \
                 tc.tile_pool(name="ropeb", bufs=2) as ropeb, \
                 tc.tile_pool(name="ropec", bufs=2) as ropec, \
                 tc.tile_pool(name="psqk", bufs=4, space="PSUM") as psqkp, \
                 tc.tile_pool(name="psv", bufs=3, space="PSUM") as psvp:
                trig_c_t = trigp.tile([128, T], bf16)
                trig_s_t = trigp.tile([128, T], bf16)
                nc.vector.memset(ones_f[:], 1.0)
                nc.vector.tensor_copy(ones_t[:], ones_f[:])

                def load_w(tile_, src):
                    for hl in range(4):
                        nc.gpsimd.dma_start(out=tile_[:, hl], in_=src[hl])

                wq_next = wpool.tile([128, 4, NCT, D], bf16, tag="w")
                # interleaved first-pass loads: wq[h0], wk[h0], trig, rest
                nc.gpsimd.dma_start(out=wq_next[:, 0], in_=wq_h[0, 0])
                for half in range(2):
                    wq_t = wq_next
                    wk_t = wpool.tile([128, 4, NCT, D], bf16, tag="w")
                    if half == 0:
                        nc.gpsimd.dma_start(out=wk_t[:, 0], in_=wk_h[0, 0])
                        nc.gpsimd.dma_start(out=trig_c_t[:], in_=trig_c[:])
                        nc.gpsimd.dma_start(out=trig_s_t[:], in_=trig_s[:])
                        for hl in range(1, 4):
                            nc.gpsimd.dma_start(out=wq_t[:, hl],
                                                in_=wq_h[0, hl])
                            nc.gpsimd.dma_start(out=wk_t[:, hl],
                                                in_=wk_h[0, hl])
                    else:
                        load_w(wk_t, wk_h[1])
                    wv_t = wpool.tile([128, NCT, 512], bf16, tag="w")
                    nc.gpsimd.dma_start(out=wv_t[:], in_=wv_h[half])
                    for tci in range(NTC):
                        ts_ = slice(tci * 512, (tci + 1) * 512)
                        if half == 0 and tci == 0:
                            xt_next = xtp.tile([128, NCT, 512], bf16,
                                               tag="xt")
                            for sp in range(4):
                                cs = slice(sp * 4, (sp + 1) * 4)
                                nc.sync.dma_start(out=xt_next[:, cs, :],
                                                  in_=xt_h[0, :, cs, :])
                        xt_t = xt_next
                        if half == 0 and tci == 2:
                            # prefetch pass-1 wq into the free 4th buffer
                            wq_next = wpool.tile([128, 4, NCT, D], bf16,
                                                 tag="w")
                            load_w(wq_next, wq_h[1])
                        for hl in range(4):
                            h = half * 4 + hl
                            lsl = slice(hl * D, (hl + 1) * D)
                            for qk in range(2):
                                w_t = wq_t if qk == 0 else wk_t
                                dst = q_all if qk == 0 else k_all
                                ps = psqkp.tile([128, 512], f32, tag="psqk")
                                for ct in range(NCT):
                                    nc.tensor.matmul(
                                        ps[:], w_t[:, hl, ct, :],
                                        xt_t[:, ct, :],
                                        start=(ct == 0), stop=(ct == NCT - 1))
                                qsb = ropea.tile([128, 512], bf16, tag="qsb")
                                nc.scalar.copy(qsb[:], ps[:])
                                qsw = ropeb.tile([128, 512], bf16, tag="qsw")
                                nc.scalar.dma_start(out=qsw[0:64, :],
                                                    in_=qsb[64:128, :])
                                nc.scalar.dma_start(out=qsw[64:128, :],
                                                    in_=qsb[0:64, :])
                                rot = ropec.tile([128, 512], bf16, tag="rot")
                                nc.vector.tensor_mul(rot[:], qsw[:],
                                                     trig_s_t[:, ts_])
                                nc.vector.tensor_mul(qsb[:], qsb[:],
                                                     trig_c_t[:, ts_])
                                nc.vector.tensor_add(dst[:, h, ts_],
                                                     qsb[:], rot[:])
                        # late prefetch of the next chunk's x^T (keeps the
                        # startup DMA burst small; 2 half-loads)
                        nci = tci + 1 if tci < NTC - 1 else (0 if half == 0
                                                            else None)
                        if nci is not None:
                            xt_next = xtp.tile([128, NCT, 512], bf16,
                                               tag="xt")
                            for sp in range(2):
                                cs = slice(sp * 8, (sp + 1) * 8)
                                nc.sync.dma_start(out=xt_next[:, cs, :],
                                                  in_=xt_h[nci, :, cs, :])
                        for sb in range(4):
                            ssl = slice(sb * 128, (sb + 1) * 128)
                            psv = psvp.tile([128, 512], f32, tag="psv")
                            for ct in range(NCT):
                                nc.tensor.matmul(
                                    psv[:], xt_t[:, ct, ssl], wv_t[:, ct, :],
                                    start=(ct == 0), stop=(ct == NCT - 1))
                            nc.scalar.copy(
                                v_all[:, tci * 4 + sb,
                                      half * 512:(half + 1) * 512],
                                psv[:])

            # ---------------- Phase 2: attention + deferred fused c_proj ---
            with tc.tile_pool(name="wppool", bufs=2) as wppool, \
                 tc.tile_pool(name="maskp", bufs=1) as maskp, \
                 tc.tile_pool(name="ptp", bufs=8) as ptp, \
                 tc.tile_pool(name="ytcp", bufs=2) as ytcp, \
                 tc.tile_pool(name="recp", bufs=2) as recp, \
                 tc.tile_pool(name="rbp", bufs=2) as rbp, \
                 tc.tile_pool(name="osbp", bufs=2) as osbp, \
                 tc.tile_pool(name="pss", bufs=2, space="PSUM") as pssp, \
                 tc.tile_pool(name="psy", bufs=2, space="PSUM") as psyp, \
                 tc.tile_pool(name="psl", bufs=2, space="PSUM") as pslp:
                masks_t = maskp.tile([128, 4, 512], bf16)
                nc.sync.dma_start(out=masks_t[:], in_=masks[:])
                wp_ts = []
                for half in range(2):
                    wp_t = wppool.tile([128, 4, C], bf16, tag="wp")
                    nc.sync.dma_start(out=wp_t[:], in_=wp_h[half])
                    wp_ts.append(wp_t)

                def emit_s_exp(tci, h):
                    ts_ = slice(tci * 512, (tci + 1) * 512)
                    jmax = 4 * tci + 3
                    pts = []
                    for jp in range((jmax + 1) // 2):
                        pss = pssp.tile([128, 2, 512], f32, tag="pss")
                        for i in range(2):
                            j = 2 * jp + i
                            nc.tensor.matmul(
                                pss[:, i, :],
                                k_all[:, h, j * 128:(j + 1) * 128],
                                q_all[:, h, ts_],
                                start=True, stop=True)
                        pt = ptp.tile([128, 2, 512], bf16, tag="pt")
                        nc.scalar.activation(
                            pt[:], pss[:],
                            mybir.ActivationFunctionType.Exp, scale=SCALE)
                        for i in range(2):
                            j = 2 * jp + i
                            if j >= 4 * tci:
                                nc.vector.tensor_mul(
                                    pt[:, i, :], pt[:, i, :],
                                    masks_t[:, j - 4 * tci, :])
                        pts.append(pt)
                    return pts

                def emit_pv_norm(tci, h, pts, ytc):
                    jmax = 4 * tci + 3
                    psy = psyp.tile([128, 512], f32, tag="psy")
                    psl = pslp.tile([1, 512], f32, tag="psl")
                    for jp in range((jmax + 1) // 2):
                        pt = pts[jp]
                        for i in range(2):
                            j = 2 * jp + i
                            nc.tensor.matmul(
                                psy[:], v_all[:, j, h * D:(h + 1) * D],
                                pt[:, i, :],
                                start=(j == 0), stop=(j == jmax))
                            nc.tensor.matmul(
                                psl[:], ones_t[:], pt[:, i, :],
                                start=(j == 0), stop=(j == jmax))
                    rec = recp.tile([1, 512], f32, tag="rec")
                    nc.vector.reciprocal(rec[:], psl[:])
                    rb = rbp.tile([128, 512], f32, tag="rb")
                    nc.gpsimd.partition_broadcast(rb[:], rec[:])
                    nc.vector.tensor_mul(ytc[:, h, :], psy[:], rb[:])

                def emit_cproj_chain(ytc_src, tci_src, c):
                    tb, hcp = c // 2, c % 2
                    tls = slice(tb * 128, (tb + 1) * 128)
                    osb = osbp.tile([128, 2, 512], f32, tag="osb")
                    pso = pssp.tile([128, 2, 512], f32, tag="pss")
                    for i in range(2):
                        ec = 2 * hcp + i
                        es = slice(ec * 512, (ec + 1) * 512)
                        for h in range(NH):
                            nc.tensor.matmul(
                                pso[:, i, :], ytc_src[:, h, tls],
                                wp_ts[h // 4][:, h % 4, es],
                                start=(h == 0), stop=(h == NH - 1))
                    nc.vector.tensor_copy(osb[:], pso[:])
                    tb_glob = tci_src * 4 + tb
                    tbs = slice(tb_glob * 128, (tb_glob + 1) * 128)
                    nc.gpsimd.dma_start(
                        out=out[tbs, hcp * 1024:(hcp + 1) * 1024], in_=osb[:])

                def emit_cproj_block(ytc_src, tci_src):
                    for c in range(8):
                        emit_cproj_chain(ytc_src, tci_src, c)

                seq = [1, 0, 2, 3]
                ytc_prev = None
                tci_prev = None
                for si, tci in enumerate(seq):
                    ytc = ytcp.tile([128, NH, 512], bf16, tag="ytc")
                    pipelined = si <= 1
                    pend = None  # (h, pts) awaiting PV in pipelined mode
                    for h in range(NH):
                        pts = emit_s_exp(tci, h)
                        if pipelined:
                            if pend is not None:
                                emit_pv_norm(tci, pend[0], pend[1], ytc)
                            pend = (h, pts)
                        else:
                            emit_pv_norm(tci, h, pts, ytc)
                        if si == 1:
                            # chunk 0 is DVE-paced: spread the deferred
                            # c_proj chains one per head to fill PE
                            if h >= 1:
                                emit_cproj_chain(ytc_prev, tci_prev, h - 1)
                        elif h == 1 and si > 0:
                            emit_cproj_block(ytc_prev, tci_prev)
                    if si == 1:
                        emit_cproj_chain(ytc_prev, tci_prev, 7)
                    if pipelined and pend is not None:
                        emit_pv_norm(tci, pend[0], pend[1], ytc)
                    ytc_prev = ytc
                    tci_prev = tci
                # tail: c_proj for the last chunk in sequence
                emit_cproj_block(ytc_prev, tci_prev)

    nc.compile()
    return nc


def _prep_inputs(x, w_attn, w_proj):
    """Build the 8 per-core input maps (host-side shard + bf16 + relayout)."""
    import ml_dtypes
    bf16 = ml_dtypes.bfloat16

    perm = np.concatenate([np.arange(0, D, 2), np.arange(1, D, 2)])

    inv = 1.0 / np.power(
        np.float32(ROPE_THETA),
        np.arange(0, D, 2, dtype=np.float32) / np.float32(D))
    pos = np.arange(T, dtype=np.float32)
    freqs = pos[:, None] * inv[None, :]  # (T, 64)
    cos_t = np.cos(freqs).T.astype(np.float32)  # (64, T)
    sin_t = np.sin(freqs).T.astype(np.float32)
    trig_c = np.concatenate([cos_t, cos_t], axis=0).astype(bf16)  # (128, T)
    trig_s = np.concatenate([-sin_t, sin_t], axis=0).astype(bf16)

    masks = np.zeros((128, 4, 512), dtype=np.float32)
    tri = (np.arange(128)[None, :] >= np.arange(128)[:, None]).astype(np.float32)
    for r in range(4):
        masks[:, r, r * 128:(r + 1) * 128] = tri
        masks[:, r, (r + 1) * 128:] = 1.0
    masks = masks.astype(bf16)

    wq_full = w_attn[:, 0:C].reshape(C, H, D)
    wk_full = w_attn[:, C:2 * C].reshape(C, H, D)

    in_maps = []
    for core in range(N_CORES):
        b, g = core // 2, core % 2
        hsel = slice(g * NH, (g + 1) * NH)
        xt_t = np.ascontiguousarray(
            x[b].reshape(NTC, 512, NCT, 128).transpose(0, 3, 2, 1)
        ).astype(bf16)
        wqc = wq_full[:, hsel, :][:, :, perm].reshape(C, NH * D)
        wkc = wk_full[:, hsel, :][:, :, perm].reshape(C, NH * D)
        wq_t = np.ascontiguousarray(
            wqc.reshape(NCT, 128, 2, 4, D).transpose(2, 3, 1, 0, 4)
        ).astype(bf16)
        wk_t = np.ascontiguousarray(
            wkc.reshape(NCT, 128, 2, 4, D).transpose(2, 3, 1, 0, 4)
        ).astype(bf16)
        wvc = w_attn[:, 2 * C + g * NH * D: 2 * C + (g + 1) * NH * D]
        wv_t = np.ascontiguousarray(
            wvc.reshape(NCT, 128, 2, 512).transpose(2, 1, 0, 3)).astype(bf16)
        wpc = w_proj[g * NH * D:(g + 1) * NH * D, :]
        wp_t = np.ascontiguousarray(
            wpc.reshape(2, 4, 128, C).transpose(0, 2, 1, 3)).astype(bf16)
        in_maps.append({
            "xt_h": xt_t,
            "wq_h": wq_t,
            "wk_h": wk_t,
            "wv_h": wv_t,
            "wp_h": wp_t,
            "trig_c": trig_c,
            "trig_s": trig_s,
            "masks": masks,
        })
    return in_maps


def _get_module():
    if "nc" not in _CACHE:
        _CACHE["nc"] = _build_module()
    return _CACHE["nc"]


def run_sharded(x, w_attn, w_proj, trace=False):
    """Run on 8 cores; returns BassKernelResults."""
    from concourse.bass_utils import run_bass_kernel_spmd
    nc = _get_module()
    in_maps = _prep_inputs(np.asarray(x), np.asarray(w_attn), np.asarray(w_proj))
    res = run_bass_kernel_spmd(nc, in_maps, core_ids=list(range(N_CORES)),
                               trace=trace)
    return res


def kernel(x, w_attn, w_proj):
    x = np.asarray(x, dtype=np.float32)
    res = run_sharded(x, w_attn, w_proj, trace=False)
    outs = [r["out"] for r in res.results]
    full = np.empty((B, T, C), dtype=np.float32)
    for b in range(B):
        full[b] = outs[2 * b].astype(np.float32) + outs[2 * b + 1].astype(np.float32)
    return full
